# revision 1
# baseline (speedup 1.0000x reference)
"""Trainium2 Bass kernel for nn_Attention_30760555774660 (stacked attention VQA net).

Sharding: data-parallel over batch, 256 -> 8 cores x 32. Weights replicated.

Per-core plan (B=32 local batch, S=196, D=1024, A=512, O=3000):
  - img_b [196,1024] DMA'd once, PE-transposed (identity matmuls) into
    imgT_b [128, 8, 196] (d-on-partitions) for the two projections.
  - Projections img_b @ W_ia{1,2} run as float32r (full-rate fp32) matmuls,
    N=512, accumulating 8 K-chunks in PSUM.
  - The broadcast add of the q-projection row is folded into the same PSUM
    accumulation with a one-hot selector matmul (K=32).
  - tanh on ScalarE; logits via DVE tensor_tensor_reduce against
    partition-broadcast Wp; softmax batched per group of 4 batch elems
    ([4,196] rows after a PE transpose of the logit columns).
  - vI = pi @ img via matmul with pi column stationary, natural img moving.
  - u1/u2 kept transposed (u1T/u2T [128, 8, 32]) so the q-projection of
    block 2 and the final FC need no extra transposes.
  - Final FC streams W_fc [128,500] tiles against stationary u2T.
"""

import os
import sys

import numpy as np

if "/opt/trn_rl_repo" not in sys.path:
    sys.path.insert(0, "/opt/trn_rl_repo")

B_FULL = 256
N_CORES = 8
B = B_FULL // N_CORES  # 32
S = 196
D = 1024
A = 512
O = 3000
G = 4  # softmax group size
NG = B // G
DC = D // 128  # 8 d-chunks
S_CHUNKS = [(0, 128), (128, 68)]
OC = 6
ON = O // OC  # 500

_nc_cache = None


def _build_nc():
    import concourse.bacc as bacc
    import concourse.tile as tile
    from concourse import mybir

    f32 = mybir.dt.float32
    f32r = mybir.dt.float32r
    Tanh = mybir.ActivationFunctionType.Tanh
    Exp = mybir.ActivationFunctionType.Exp
    mult = mybir.AluOpType.mult
    add = mybir.AluOpType.add

    nc = bacc.Bacc("TRN2", target_bir_lowering=False)

    img_h = nc.dram_tensor("img", [B, S, D], f32r, kind="ExternalInput")
    ques_h = nc.dram_tensor("ques", [B, D], f32, kind="ExternalInput")
    wia1_h = nc.dram_tensor("W_ia1", [D, A], f32r, kind="ExternalInput")
    wqa1_h = nc.dram_tensor("W_qa1", [D, A], f32r, kind="ExternalInput")
    bqa1_h = nc.dram_tensor("b_qa1", [A], f32, kind="ExternalInput")
    wp1_h = nc.dram_tensor("Wp1", [A], f32, kind="ExternalInput")
    wia2_h = nc.dram_tensor("W_ia2", [D, A], f32r, kind="ExternalInput")
    wqa2_h = nc.dram_tensor("W_qa2", [D, A], f32r, kind="ExternalInput")
    bqa2_h = nc.dram_tensor("b_qa2", [A], f32, kind="ExternalInput")
    wp2_h = nc.dram_tensor("Wp2", [A], f32, kind="ExternalInput")
    wfc_h = nc.dram_tensor("W_fc", [D, O], f32r, kind="ExternalInput")
    bfc_h = nc.dram_tensor("b_fc", [O], f32, kind="ExternalInput")
    oneh_h = nc.dram_tensor("ONEHOTS", [B, B, 128], f32r, kind="ExternalInput")
    ident_h = nc.dram_tensor("IDENT", [128, 128], f32r, kind="ExternalInput")
    identf_h = nc.dram_tensor("IDENTF", [128, 128], f32, kind="ExternalInput")
    score_h = nc.dram_tensor("score", [B, O], f32, kind="ExternalOutput")

    import bass_rust  # noqa: F401
    import concourse.bass as bass  # noqa: F401

    def bcast_ap(h, n_part, free_n):
        # partition-stride-0 broadcast read of a 1-D dram tensor
        ap = h[:]
        return bass.AP(tensor=ap.tensor, offset=ap.offset, ap=[[0, n_part]] + ap.ap)

    with tile.TileContext(nc) as tc:
        with (
            tc.tile_pool(name="const", bufs=1) as const,
            tc.tile_pool(name="imgn", bufs=5) as imgn_p,
            tc.tile_pool(name="imgt", bufs=3) as imgt_p,
            tc.tile_pool(name="p2sb", bufs=5) as p2sb_p,
            tc.tile_pool(name="work", bufs=2) as work,
            tc.tile_pool(name="work1", bufs=1) as work1,
            tc.tile_pool(name="wstream", bufs=3) as wstream,
            tc.tile_pool(name="pst", bufs=5, space="PSUM") as pst,
            tc.tile_pool(name="psp", bufs=3, space="PSUM") as psp,
        ):
            # ---------------- constants ----------------
            ident = const.tile([128, 128], f32r)
            nc.sync.dma_start(out=ident, in_=ident_h[:, :])
            identf = const.tile([128, 128], f32)
            nc.sync.dma_start(out=identf, in_=identf_h[:, :])
            oneh = const.tile([B, B, 128], f32r)
            nc.sync.dma_start(out=oneh, in_=oneh_h[:, :, :])
            wia1 = const.tile([128, DC, A], f32r)
            nc.sync.dma_start(out=wia1, in_=wia1_h[:, :].rearrange("(c p) a -> p c a", p=128))
            wia2 = const.tile([128, DC, A], f32r)
            nc.sync.dma_start(out=wia2, in_=wia2_h[:, :].rearrange("(c p) a -> p c a", p=128))
            wqa2 = const.tile([128, DC, A], f32r)
            nc.sync.dma_start(out=wqa2, in_=wqa2_h[:, :].rearrange("(c p) a -> p c a", p=128))
            bqa1b = const.tile([B, A], f32)
            nc.gpsimd.dma_start(out=bqa1b, in_=bcast_ap(bqa1_h, B, A))
            bqa2b = const.tile([B, A], f32)
            nc.gpsimd.dma_start(out=bqa2b, in_=bcast_ap(bqa2_h, B, A))
            wp1b = const.tile([128, A], f32)
            nc.gpsimd.dma_start(out=wp1b, in_=bcast_ap(wp1_h, 128, A))
            wp2b = const.tile([128, A], f32)
            nc.gpsimd.dma_start(out=wp2b, in_=bcast_ap(wp2_h, 128, A))
            quesA = const.tile([B, D], f32)
            nc.sync.dma_start(out=quesA, in_=ques_h[:, :])

            quesT = const.tile([128, DC, B], f32r)
            QP1 = const.tile([B, A], f32r)
            QP2 = const.tile([B, A], f32r)
            u1T = const.tile([128, DC, B], f32r)
            u2T = const.tile([128, DC, B], f32r)
            nc.vector.memset(QP2[:, :].bitcast(f32), 0.0)
            nc.vector.memset(u1T[:, :, :].bitcast(f32), 0.0)

            def r(ap):
                return ap

            # quesT[p, c, b] = ques[b, c*128+p]
            for c in range(DC):
                pt = pst.tile([128, B], f32, tag="tr")
                nc.tensor.transpose(pt, quesA[:, c * 128 : (c + 1) * 128], identf[0:B, 0:B])
                nc.vector.tensor_copy(quesT[:, c, :], pt)

            # QP1 = ques @ W_qa1 + b_qa1   [32, 512]
            qp_ps = psp.tile([B, A], f32, tag="pp")
            for c in range(DC):
                wq = wstream.tile([128, A], f32r, tag="ws")
                nc.sync.dma_start(out=wq, in_=wqa1_h[c * 128 : (c + 1) * 128, :])
                nc.tensor.matmul(qp_ps, r(quesT[:, c, :]), r(wq), start=(c == 0), stop=(c == DC - 1))
            nc.vector.tensor_add(QP1, qp_ps, bqa1b)

            imgN = {}
            imgT = {}
            p2sb = {}
            Lc1 = {}
            Lc2 = {}

            def load_and_proj(b):
                """DMA img_b, transpose, run both projections. Block-1 proj
                gets the QP1 broadcast folded in and goes through tanh+logits;
                block-2 proj parks in SBUF."""
                inb = imgn_p.tile([128, 2, D], f32r, tag="imgn")
                imgN[b] = inb
                nc.sync.dma_start(out=inb[:, 0, :], in_=img_h[b : b + 1, 0:128, :].rearrange("o s d -> (o s) d"))
                nc.sync.dma_start(out=inb[0:68, 1, :], in_=img_h[b : b + 1, 128:196, :].rearrange("o s d -> (o s) d"))
                itb = imgt_p.tile([128, DC, S], f32r, tag="imgt")
                imgT[b] = itb
                for c in range(DC):
                    pa = pst.tile([128, 128], f32, tag="tr")
                    nc.tensor.transpose(pa, inb[0:128, 0, c * 128 : (c + 1) * 128].bitcast(f32), identf)
                    nc.vector.tensor_copy(itb[:, c, 0:128], pa)
                    pb = pst.tile([128, 128], f32, tag="tr")
                    nc.tensor.transpose(pb[:, 0:68], inb[0:68, 1, c * 128 : (c + 1) * 128].bitcast(f32), identf[0:68, 0:68])
                    nc.vector.tensor_copy(itb[:, c, 128:196], pb[:, 0:68])
                # block-1 projection + QP1 broadcast + tanh + logits
                lc = work.tile([128, 2, G], f32, tag="lc1")
                if b % G == 0:
                    Lc1[b // G] = lc
                lc = Lc1[b // G]
                for si, (s0, sl) in enumerate(S_CHUNKS):
                    pp = psp.tile([128, A], f32, tag="pp")
                    for c in range(DC):
                        nc.tensor.matmul(
                            pp[0:sl, :], r(itb[0:128, c, s0 : s0 + sl]), r(wia1[:, c, :]),
                            start=(c == 0), stop=False,
                        )
                    nc.tensor.matmul(pp[0:sl, :], r(oneh[:, b, 0:sl]), r(QP1), start=False, stop=True)
                    ha = work.tile([128, A], f32, tag="ha")
                    nc.scalar.activation(ha[0:sl], pp[0:sl], Tanh)
                    prod = work.tile([128, A], f32, tag="prod")
                    nc.vector.tensor_mul(prod[0:sl], ha[0:sl], wp1b[0:sl])
                    nc.vector.tensor_reduce(
                        lc[0:sl, si, b % G : b % G + 1], prod[0:sl],
                        axis=mybir.AxisListType.X, op=add,
                    )
                # block-2 projection -> SBUF
                p2 = p2sb_p.tile([128, 2, A], f32, tag="p2sb")
                p2sb[b] = p2
                for si, (s0, sl) in enumerate(S_CHUNKS):
                    pp = psp.tile([128, A], f32, tag="pp")
                    for c in range(DC):
                        nc.tensor.matmul(
                            pp[0:sl, :], r(itb[0:128, c, s0 : s0 + sl]), r(wia2[:, c, :]),
                            start=(c == 0), stop=(c == DC - 1),
                        )
                    nc.vector.tensor_copy(p2[0:sl, si, :], pp[0:sl])

            def softmax_and_pi(lc, tagp):
                """logit columns [128, 2, G] -> PI [G, 196] softmax rows."""
                LT = work.tile([G, S], f32, tag="LT")
                pa = pst.tile([128, 128], f32, tag="tr")
                nc.tensor.transpose(pa[0:G, :], lc[:, 0, :], identf)
                nc.vector.tensor_copy(LT[:, 0:128], pa[0:G, :])
                pb = pst.tile([128, 128], f32, tag="tr")
                nc.tensor.transpose(pb[0:G, 0:68], lc[0:68, 1, :], identf[0:68, 0:68])
                nc.vector.tensor_copy(LT[:, 128:196], pb[0:G, 0:68])
                E = work.tile([G, S], f32, tag="E")
                Z = work.tile([G, 1], f32, tag="Z")
                nc.scalar.activation(E, LT, Exp, accum_out=Z)
                R = work.tile([G, 1], f32, tag="R")
                nc.vector.reciprocal(R, Z)
                PI = work.tile([G, S], f32, tag=tagp)
                nc.vector.tensor_scalar_mul(PI, E, R)
                return PI

            def weighted_sum_add(PI, g, other, out_tag, pool=None):
                """out[bb,:] = vI_bb + other[bb,:], via masked-column pi^T
                stationaries accumulating the whole group in one PSUM tile."""
                piTm = work.tile([128, 2, G, G], f32r, tag="piTm")
                for bb in range(G):
                    PIm = work.tile([G, S], f32, tag="PIm")
                    nc.vector.tensor_scalar_mul(PIm, PI, oneh[0:G, bb, 0:1].bitcast(f32))
                    pc = pst.tile([128, 128], f32, tag="tr")
                    nc.tensor.transpose(pc[:, 0:G], PIm[:, 0:128], identf[0:G, 0:G])
                    nc.vector.tensor_copy(piTm[:, 0, bb, :], pc[:, 0:G])
                    pd = pst.tile([128, 128], f32, tag="tr")
                    nc.tensor.transpose(pd[0:68, 0:G], PIm[:, 128:196], identf[0:G, 0:G])
                    nc.vector.tensor_copy(piTm[0:68, 1, bb, :], pd[0:68, 0:G])
                out = (pool or work).tile([G, D], f32, tag=out_tag)
                for h in range(2):
                    vp = psp.tile([G, A], f32, tag="pp")
                    k = 0
                    for bb in range(G):
                        inb = imgN[g * G + bb]
                        for si, (s0, sl) in enumerate(S_CHUNKS):
                            nc.tensor.matmul(
                                vp, r(piTm[0:sl, si, bb, :]), r(inb[0:sl, si, h * A : (h + 1) * A]),
                                start=(k == 0), stop=(k == 2 * G - 1),
                            )
                            k += 1
                    nc.vector.tensor_add(out[:, h * A : (h + 1) * A], vp, other[:, h * A : (h + 1) * A])
                return out

            NG_RUN = int(os.environ.get("NG_RUN", str(NG)))
            for g in range(NG_RUN):
                g4 = g * G
                for bb in range(G):
                    load_and_proj(g4 + bb)
                # ---- block 1 softmax / vI / u1 ----
                PI1 = softmax_and_pi(Lc1[g], "PI1")
                qn = work1.tile([G, D], f32, tag="qn")
                nc.sync.dma_start(out=qn, in_=ques_h[g4 : g4 + G, :])
                u1g = weighted_sum_add(PI1, g, qn, "u1g")
                for c in range(DC):
                    pt = pst.tile([128, 128], f32, tag="tr")
                    nc.tensor.transpose(pt[:, 0:G], u1g[:, c * 128 : (c + 1) * 128], identf[0:G, 0:G])
                    nc.vector.tensor_copy(u1T[:, c, g4 : g4 + G], pt[:, 0:G])
                # qp2 for this group (M=32, only rows g4:g4+G fresh)
                q2p = psp.tile([B, A], f32, tag="pp")
                for c in range(DC):
                    nc.tensor.matmul(q2p, r(u1T[:, c, :]), r(wqa2[:, c, :]), start=(c == 0), stop=(c == DC - 1))
                nc.vector.tensor_add(QP2, q2p, bqa2b)
                # ---- block 2 ----
                lc2 = work.tile([128, 2, G], f32, tag="lc2")
                Lc2[g] = lc2
                for bb in range(G):
                    b = g4 + bb
                    for si, (s0, sl) in enumerate(S_CHUNKS):
                        pp = psp.tile([128, A], f32, tag="pp")
                        nc.tensor.matmul(pp[0:sl, :], r(oneh[:, b, 0:sl]), r(QP2), start=True, stop=True)
                        ha2 = work.tile([128, A], f32, tag="ha")
                        nc.vector.tensor_add(ha2[0:sl], pp[0:sl], p2sb[b][0:sl, si, :])
                        nc.scalar.activation(ha2[0:sl], ha2[0:sl], Tanh)
                        prod = work.tile([128, A], f32, tag="prod")
                        nc.vector.tensor_mul(prod[0:sl], ha2[0:sl], wp2b[0:sl])
                        nc.vector.tensor_reduce(
                            lc2[0:sl, si, bb : bb + 1], prod[0:sl],
                            axis=mybir.AxisListType.X, op=add,
                        )
                PI2 = softmax_and_pi(lc2, "PI2")
                u2g = weighted_sum_add(PI2, g, u1g, "u2g", pool=work1)
                for c in range(DC):
                    pt = pst.tile([128, 128], f32, tag="tr")
                    nc.tensor.transpose(pt[:, 0:G], u2g[:, c * 128 : (c + 1) * 128], identf[0:G, 0:G])
                    nc.vector.tensor_copy(u2T[:, c, g4 : g4 + G], pt[:, 0:G])

            # ---------------- final FC ----------------
            for n in range(OC):
                fp = psp.tile([B, ON], f32, tag="pp")
                for c in range(DC):
                    wf = wstream.tile([128, ON], f32r, tag="ws")
                    nc.sync.dma_start(out=wf, in_=wfc_h[c * 128 : (c + 1) * 128, n * ON : (n + 1) * ON])
                    nc.tensor.matmul(fp, r(u2T[:, c, :]), r(wf), start=(c == 0), stop=(c == DC - 1))
                bf = work1.tile([B, ON], f32, tag="bf")
                nc.gpsimd.dma_start(
                    out=bf,
                    in_=_slice_bcast(bfc_h, B, n * ON, ON),
                )
                sc = work.tile([B, ON], f32, tag="sc")
                nc.vector.tensor_add(sc, fp, bf)
                nc.sync.dma_start(out=score_h[:, n * ON : (n + 1) * ON], in_=sc)

    nc.compile()
    return nc


def _slice_bcast(h, n_part, off, n):
    import concourse.bass as bass

    ap = h[off : off + n]
    return bass.AP(tensor=ap.tensor, offset=ap.offset, ap=[[0, n_part]] + ap.ap)


def _get_nc():
    global _nc_cache
    if _nc_cache is None:
        _nc_cache = _build_nc()
    return _nc_cache


def _make_in_maps(inputs):
    onehots = np.ascontiguousarray(
        np.repeat(np.eye(B, dtype=np.float32)[:, :, None], 128, axis=2)
    )
    ident = np.eye(128, dtype=np.float32)
    shared = {
        "W_ia1": np.ascontiguousarray(inputs["W_ia1"], np.float32),
        "W_qa1": np.ascontiguousarray(inputs["W_qa1"], np.float32),
        "b_qa1": np.ascontiguousarray(inputs["b_qa1"], np.float32),
        "Wp1": np.ascontiguousarray(inputs["Wp1"], np.float32),
        "W_ia2": np.ascontiguousarray(inputs["W_ia2"], np.float32),
        "W_qa2": np.ascontiguousarray(inputs["W_qa2"], np.float32),
        "b_qa2": np.ascontiguousarray(inputs["b_qa2"], np.float32),
        "Wp2": np.ascontiguousarray(inputs["Wp2"], np.float32),
        "W_fc": np.ascontiguousarray(inputs["W_fc"], np.float32),
        "b_fc": np.ascontiguousarray(inputs["b_fc"], np.float32),
        "ONEHOTS": onehots,
        "IDENT": ident,
        "IDENTF": ident,
    }
    in_maps = []
    for c in range(N_CORES):
        sl = slice(c * B, (c + 1) * B)
        m = dict(shared)
        m["img"] = np.ascontiguousarray(inputs["img_feat"][sl], np.float32)
        m["ques"] = np.ascontiguousarray(inputs["ques_feat"][sl], np.float32)
        in_maps.append(m)
    return in_maps


def kernel_run(inputs, trace=False):
    from concourse.bass_utils import run_bass_kernel_spmd

    nc = _get_nc()
    in_maps = _make_in_maps(inputs)
    res = run_bass_kernel_spmd(nc, in_maps, core_ids=list(range(N_CORES)), trace=trace)
    out = np.concatenate([r["score"] for r in res.results], axis=0)
    return out, res


def kernel(**inputs):
    out, _ = kernel_run(inputs)
    return out



# revision 4
# speedup vs baseline: 2.0790x; 2.0790x over previous
"""Trainium2 Bass kernel for nn_Attention_30760555774660 (stacked attention VQA net).

Sharding: data-parallel over batch, 256 -> 8 cores x 32. Weights replicated.

v2 design (per core: B=32, S=196, D=1024, A=512, O=3000):
  - All big matmuls in bf16 (tolerance gate is 2e-2; bf16 lands ~5e-3).
  - Host supplies img in BOTH layouts: native [b, s-chunks, d] for the
    attention-weighted sums, and transposed [b, d-part, c, s] for the
    projections. No on-device transposes of img at all.
  - Projections run transposed: projT[a, s] = W_ia[:, a-chunk].T @ imgT,
    W chunks as the stationary operand, out [128a, 196] PSUM.
  - q-projection broadcast + b_qa fold into the tanh as the per-partition
    activation bias (QP1T/QP2T [a-part, b]).
  - logits = Wp.T @ haT on PE (M=1 matvec), softmax per b on one partition,
    E transposed back to [s, 1] via tiny PE transposes into a pre-masked
    [s, 8] group tile; vI for 8 b's accumulates in one [8, 2, 512] PSUM.
  - u = vI*R + prev via one fused scalar_tensor_tensor per 512-chunk.
  - Final FC streams W_fc bf16 tiles against stationary u2T columns, with
    b_fc folded in via a K=1 ones-row matmul.
"""

import sys

import numpy as np

if "/opt/trn_rl_repo" not in sys.path:
    sys.path.insert(0, "/opt/trn_rl_repo")

B_FULL = 256
N_CORES = 8
B = B_FULL // N_CORES  # 32
S = 196
D = 1024
A = 512
O = 3000
DC = 8  # d chunks of 128
AC = 4  # a chunks of 128
OB = 8  # batch group (oct)
NOCT = B // OB  # 4
ON = 500
OC = 6
S_CHUNKS = ((0, 128), (1, 68))

_nc_cache = None


def _build_nc():
    import concourse.bacc as bacc
    import concourse.tile as tile
    from concourse import mybir

    f32 = mybir.dt.float32
    bf16 = mybir.dt.bfloat16
    Tanh = mybir.ActivationFunctionType.Tanh
    Exp = mybir.ActivationFunctionType.Exp
    mult = mybir.AluOpType.mult
    add = mybir.AluOpType.add

    nc = bacc.Bacc("TRN2", target_bir_lowering=False)

    imgN_h = nc.dram_tensor("imgN", [B, 128, 2, D], bf16, kind="ExternalInput")
    imgT_h = nc.dram_tensor("imgT", [B, 128, DC, S], bf16, kind="ExternalInput")
    quesN_h = nc.dram_tensor("quesN", [B, D], f32, kind="ExternalInput")
    quesT_h = nc.dram_tensor("quesT", [128, DC, B], bf16, kind="ExternalInput")
    wia1_h = nc.dram_tensor("wia1", [128, DC, A], bf16, kind="ExternalInput")
    wia2_h = nc.dram_tensor("wia2", [128, DC, A], bf16, kind="ExternalInput")
    wqa1_h = nc.dram_tensor("wqa1", [128, DC, A], bf16, kind="ExternalInput")
    wqa2_h = nc.dram_tensor("wqa2", [128, DC, A], bf16, kind="ExternalInput")
    wfc_h = nc.dram_tensor("wfc", [128, DC, O], bf16, kind="ExternalInput")
    bfc_h = nc.dram_tensor("bfc", [1, O], bf16, kind="ExternalInput")
    wp1_h = nc.dram_tensor("wp1", [128, AC], bf16, kind="ExternalInput")
    wp2_h = nc.dram_tensor("wp2", [128, AC], bf16, kind="ExternalInput")
    bqa1T_h = nc.dram_tensor("bqa1T", [128, AC], f32, kind="ExternalInput")
    bqa2T_h = nc.dram_tensor("bqa2T", [128, AC], f32, kind="ExternalInput")
    onesb_h = nc.dram_tensor("onesb", [1, B], bf16, kind="ExternalInput")
    identf_h = nc.dram_tensor("identf", [128, 128], f32, kind="ExternalInput")
    identb_h = nc.dram_tensor("identb", [128, 128], bf16, kind="ExternalInput")
    score_h = nc.dram_tensor("score", [B, O], f32, kind="ExternalOutput")

    from contextlib import ExitStack

    with tile.TileContext(nc) as tc:
        with ExitStack() as stack:
            pool = lambda **kw: stack.enter_context(tc.tile_pool(**kw))
            const = pool(name="const", bufs=1)
            wqp = pool(name="wq", bufs=2)
            imgn_p = pool(name="imgn", bufs=12)
            imgt_p = pool(name="imgt", bufs=12)
            hap = pool(name="ha", bufs=4)
            parkp = pool(name="park", bufs=10)
            ep = pool(name="ep", bufs=4)
            etp = pool(name="etp", bufs=3)
            zp = pool(name="zp", bufs=4)
            rp = pool(name="rp", bufs=4)
            qpool = pool(name="qp", bufs=2)
            upool = pool(name="up", bufs=3)
            u1tp = pool(name="u1tp", bufs=2)
            qp2tp = pool(name="qp2tp", bufs=2)
            qpsp = pool(name="qps", bufs=2)
            wfp = pool(name="wf", bufs=16)
            scp = pool(name="sc", bufs=2)
            pps = pool(name="psproj", bufs=3, space="PSUM")
            psf = pool(name="pssmf", bufs=2, space="PSUM")
            psb = pool(name="pssmb", bufs=1, space="PSUM")
            pvi = pool(name="psvi", bufs=1, space="PSUM")
            # ---------------- constants ----------------
            identf = const.tile([128, 128], f32, tag="identf")
            nc.sync.dma_start(out=identf, in_=identf_h[:, :])
            identb = const.tile([128, 128], bf16, tag="identb")
            nc.sync.dma_start(out=identb, in_=identb_h[:, :])
            onesb = const.tile([1, B], bf16, tag="onesb")
            nc.sync.dma_start(out=onesb, in_=onesb_h[:, :])
            wia1 = const.tile([128, DC, A], bf16, tag="wia1")
            nc.sync.dma_start(out=wia1, in_=wia1_h[:, :, :])
            wia2 = const.tile([128, DC, A], bf16, tag="wia2")
            nc.sync.dma_start(out=wia2, in_=wia2_h[:, :, :])
            wqa2 = const.tile([128, DC, A], bf16, tag="wqa2")
            nc.sync.dma_start(out=wqa2, in_=wqa2_h[:, :, :])
            wp1 = const.tile([128, AC], bf16, tag="wp1")
            nc.sync.dma_start(out=wp1, in_=wp1_h[:, :])
            wp2 = const.tile([128, AC], bf16, tag="wp2")
            nc.sync.dma_start(out=wp2, in_=wp2_h[:, :])
            bqa1T = const.tile([128, AC], f32, tag="bqa1T")
            nc.sync.dma_start(out=bqa1T, in_=bqa1T_h[:, :])
            bqa2T = const.tile([128, AC], f32, tag="bqa2T")
            nc.sync.dma_start(out=bqa2T, in_=bqa2T_h[:, :])
            bfc = const.tile([1, O], bf16, tag="bfc")
            nc.sync.dma_start(out=bfc, in_=bfc_h[:, :])
            quesT = const.tile([128, DC, B], bf16, tag="quesT")
            nc.sync.dma_start(out=quesT, in_=quesT_h[:, :, :])
            QP1T = const.tile([128, AC, B], f32, tag="QP1T")
            u2T = const.tile([128, DC, B], bf16, tag="u2T")

            # ---------------- QP1T = (ques @ W_qa1 + b_qa1)^T ----------------
            qp1_ps = psf.tile([B, A], f32, tag="smf")
            for d in range(DC):
                wqs = wqp.tile([128, A], bf16, tag="wq")
                nc.sync.dma_start(out=wqs, in_=wqa1_h[:, d, :])
                nc.tensor.matmul(
                    qp1_ps, quesT[:, d, :], wqs, start=(d == 0), stop=(d == DC - 1)
                )
            qp1s = qpsp.tile([B, A], f32, tag="qps")
            nc.scalar.copy(qp1s, qp1_ps)
            for c in range(AC):
                pt = psf.tile([128, B], f32, tag="smf")
                nc.tensor.transpose(
                    pt, qp1s[:, c * 128 : (c + 1) * 128], identf[0:B, 0:B]
                )
                nc.vector.tensor_scalar_add(QP1T[:, c, :], pt, bqa1T[:, c : c + 1])

            def proj_block(itb, wia, ppts):
                """Two projection PSUM tiles [128, 2, 196] (a01, a23)."""
                for half in range(2):
                    ppt = ppts[half]
                    for a_loc in range(2):
                        a = half * 2 + a_loc
                        for d in range(DC):
                            nc.tensor.matmul(
                                ppt[:, a_loc, :],
                                wia[:, d, a * 128 : (a + 1) * 128],
                                itb[:, d, :],
                                start=(d == 0),
                                stop=(d == DC - 1),
                            )

            def logits_softmax(ha, wp, et, z, bb):
                """logits -> exp -> E^T columns into the pre-masked group tile."""
                lg = psf.tile([1, S], f32, tag="smf")
                for c in range(AC):
                    nc.tensor.matmul(
                        lg, wp[:, c : c + 1], ha[:, c, :], start=(c == 0), stop=(c == AC - 1)
                    )
                E = ep.tile([1, S], bf16, tag="E")
                nc.scalar.activation(E, lg, Exp, accum_out=z[0:1, bb : bb + 1])
                for si, sl in S_CHUNKS:
                    pt = psb.tile([128, 1], bf16, tag="smb")
                    nc.tensor.transpose(
                        pt[0:sl, :], E[0:1, si * 128 : si * 128 + sl], identb[0:1, 0:1]
                    )
                    nc.vector.tensor_copy(et[0:sl, si, bb, bb : bb + 1], pt[0:sl, :])

            def group_vI_u(et, z, inbs, other):
                """vI for 8 b's + fused u = vI*R + other. Returns u [8, 1024] f32."""
                ztp = psf.tile([OB, 1], f32, tag="smf")
                nc.tensor.transpose(ztp[0:OB, :], z[0:1, 0:OB], identf[0:1, 0:1])
                R = rp.tile([OB, 1], f32, tag="R")
                nc.vector.reciprocal(R, ztp[0:OB, :])
                vip = pvi.tile([OB, 2, A], f32, tag="vi")
                for bb in range(OB):
                    for si, sl in S_CHUNKS:
                        for n in range(2):
                            nc.tensor.matmul(
                                vip[0:OB, n, :],
                                et[0:sl, si, bb, :],
                                inbs[bb][0:sl, si, n * A : (n + 1) * A],
                                start=(bb == 0 and si == 0),
                                stop=(bb == OB - 1 and si == 1),
                            )
                u = upool.tile([OB, D], f32, tag="u")
                for n in range(2):
                    nc.vector.scalar_tensor_tensor(
                        u[0:OB, n * A : (n + 1) * A],
                        vip[0:OB, n, :],
                        R[0:OB, 0:1],
                        other[0:OB, n * A : (n + 1) * A],
                        op0=mult,
                        op1=add,
                    )
                return u

            # ---------------- main loop over octs ----------------
            for g in range(NOCT):
                g8 = g * OB
                q8 = qpool.tile([OB, D], f32, tag="q8")
                nc.sync.dma_start(out=q8, in_=quesN_h[g8 : g8 + OB, :])
                inbs = {}
                itbs = {}
                for bb in range(OB):
                    b = g8 + bb
                    inb = imgn_p.tile([128, 2, D], bf16, tag="imgn")
                    nc.sync.dma_start(
                        out=inb,
                        in_=imgN_h[b : b + 1, :, :, :].rearrange("o p k d -> (o p) k d"),
                    )
                    inbs[bb] = inb
                    itb = imgt_p.tile([128, DC, S], bf16, tag="imgt")
                    nc.sync.dma_start(
                        out=itb,
                        in_=imgT_h[b : b + 1, :, :, :].rearrange("o p c s -> (o p) c s"),
                    )
                    itbs[bb] = itb

                # block 1 per-b: proj -> tanh -> logits -> exp -> E^T
                et1 = etp.tile([128, 2, OB, OB], bf16, tag="et")
                nc.vector.memset(et1, 0.0)
                z1 = zp.tile([1, OB], f32, tag="z")
                ha1s = {}
                for bb in range(OB):
                    ppts = [
                        pps.tile([128, 2, S], f32, tag="proj", name=f"pj1a_{g}_{bb}"),
                        pps.tile([128, 2, S], f32, tag="proj", name=f"pj1b_{g}_{bb}"),
                    ]
                    proj_block(itbs[bb], wia1, ppts)
                    ha = hap.tile([128, AC, S], bf16, tag="ha")
                    for c in range(AC):
                        nc.scalar.activation(
                            ha[:, c, :],
                            ppts[c // 2][:, c % 2, :],
                            Tanh,
                            bias=QP1T[:, c, g8 + bb : g8 + bb + 1],
                        )
                    ha1s[bb] = ha
                    logits_softmax(ha, wp1, et1, z1, bb)

                # block 2 projections parked to SBUF (independent of u1)
                parks = {}
                for bb in range(OB):
                    ppts = [
                        pps.tile([128, 2, S], f32, tag="proj", name=f"pj2a_{g}_{bb}"),
                        pps.tile([128, 2, S], f32, tag="proj", name=f"pj2b_{g}_{bb}"),
                    ]
                    proj_block(itbs[bb], wia2, ppts)
                    pk = parkp.tile([128, AC, S], bf16, tag="park")
                    for c in range(AC):
                        nc.scalar.copy(pk[:, c, :], ppts[c // 2][:, c % 2, :])
                    parks[bb] = pk

                # u1 = vI1 + ques
                u1 = group_vI_u(et1, z1, inbs, q8)

                # u1T (bf16) for the QP2 matvec
                u1T = u1tp.tile([128, DC, OB], bf16, tag="u1T")
                for c in range(DC):
                    pt = psf.tile([128, OB], f32, tag="smf")
                    nc.tensor.transpose(
                        pt[:, 0:OB], u1[0:OB, c * 128 : (c + 1) * 128], identf[0:OB, 0:OB]
                    )
                    nc.vector.tensor_copy(u1T[:, c, :], pt[:, 0:OB])

                # QP2T = (u1 @ W_qa2 + b_qa2)^T
                qp2_ps = psf.tile([OB, A], f32, tag="smf")
                for d in range(DC):
                    nc.tensor.matmul(
                        qp2_ps, u1T[:, d, :], wqa2[:, d, :], start=(d == 0), stop=(d == DC - 1)
                    )
                qp2s = qpsp.tile([OB, A], f32, tag="qps")
                nc.scalar.copy(qp2s, qp2_ps)
                QP2T = qp2tp.tile([128, AC, OB], f32, tag="qp2T")
                for c in range(AC):
                    pt = psf.tile([128, OB], f32, tag="smf")
                    nc.tensor.transpose(
                        pt[:, 0:OB], qp2s[0:OB, c * 128 : (c + 1) * 128], identf[0:OB, 0:OB]
                    )
                    nc.vector.tensor_scalar_add(QP2T[:, c, :], pt[:, 0:OB], bqa2T[:, c : c + 1])

                # block 2 per-b tail
                et2 = etp.tile([128, 2, OB, OB], bf16, tag="et")
                nc.vector.memset(et2, 0.0)
                z2 = zp.tile([1, OB], f32, tag="z")
                for bb in range(OB):
                    ha2 = hap.tile([128, AC, S], bf16, tag="ha")
                    for c in range(AC):
                        nc.scalar.activation(
                            ha2[:, c, :],
                            parks[bb][:, c, :],
                            Tanh,
                            bias=QP2T[:, c, bb : bb + 1],
                        )
                    logits_softmax(ha2, wp2, et2, z2, bb)

                # u2 = vI2 + u1
                u2 = group_vI_u(et2, z2, inbs, u1)
                for c in range(DC):
                    pt = psf.tile([128, OB], f32, tag="smf")
                    nc.tensor.transpose(
                        pt[:, 0:OB], u2[0:OB, c * 128 : (c + 1) * 128], identf[0:OB, 0:OB]
                    )
                    nc.vector.tensor_copy(u2T[:, c, g8 : g8 + OB], pt[:, 0:OB])

            # ---------------- final FC ----------------
            for n in range(OC):
                wfn = []
                for c in range(DC):
                    wf = wfp.tile([128, ON], bf16, tag="wf")
                    nc.sync.dma_start(out=wf, in_=wfc_h[:, c, n * ON : (n + 1) * ON])
                    wfn.append(wf)
                sp = pps.tile([B, ON], f32, tag="proj")
                for c in range(DC):
                    nc.tensor.matmul(sp, u2T[:, c, :], wfn[c], start=(c == 0), stop=False)
                nc.tensor.matmul(
                    sp, onesb[0:1, :], bfc[0:1, n * ON : (n + 1) * ON], start=False, stop=True
                )
                sc = scp.tile([B, ON], f32, tag="sc")
                nc.scalar.copy(sc, sp)
                nc.sync.dma_start(out=score_h[:, n * ON : (n + 1) * ON], in_=sc)

    nc.compile()
    return nc


def _get_nc():
    global _nc_cache
    if _nc_cache is None:
        _nc_cache = _build_nc()
    return _nc_cache


def _make_in_maps(inputs):
    import ml_dtypes

    bf = ml_dtypes.bfloat16

    def f32a(x):
        return np.ascontiguousarray(np.asarray(x), np.float32)

    def wchunk(w):  # [D, N] -> [128, DC, N]
        w = f32a(w)
        return np.ascontiguousarray(
            w.reshape(DC, 128, w.shape[1]).transpose(1, 0, 2).astype(bf)
        )

    def acolT(v, dt):  # [A] -> [128, AC]
        return np.ascontiguousarray(f32a(v).reshape(AC, 128).T.astype(dt))

    img = f32a(inputs["img_feat"])  # [256, 196, 1024]
    ques = f32a(inputs["ques_feat"])  # [256, 1024]

    imgN = np.zeros((B_FULL, 128, 2, D), np.float32)
    imgN[:, :, 0, :] = img[:, 0:128, :]
    imgN[:, 0:68, 1, :] = img[:, 128:196, :]
    imgN = np.ascontiguousarray(imgN.astype(bf))
    imgT = np.ascontiguousarray(
        img.reshape(B_FULL, S, DC, 128).transpose(0, 3, 2, 1).astype(bf)
    )

    shared = {
        "wia1": wchunk(inputs["W_ia1"]),
        "wia2": wchunk(inputs["W_ia2"]),
        "wqa1": wchunk(inputs["W_qa1"]),
        "wqa2": wchunk(inputs["W_qa2"]),
        "wfc": wchunk(inputs["W_fc"]),
        "bfc": np.ascontiguousarray(f32a(inputs["b_fc"]).reshape(1, O).astype(bf)),
        "wp1": acolT(inputs["Wp1"], bf),
        "wp2": acolT(inputs["Wp2"], bf),
        "bqa1T": acolT(inputs["b_qa1"], np.float32),
        "bqa2T": acolT(inputs["b_qa2"], np.float32),
        "onesb": np.ones((1, B), bf),
        "identf": np.eye(128, dtype=np.float32),
        "identb": np.eye(128, dtype=np.float32).astype(bf),
    }
    in_maps = []
    for core in range(N_CORES):
        sl = slice(core * B, (core + 1) * B)
        m = dict(shared)
        m["imgN"] = imgN[sl]
        m["imgT"] = imgT[sl]
        m["quesN"] = np.ascontiguousarray(ques[sl])
        m["quesT"] = np.ascontiguousarray(
            ques[sl].reshape(B, DC, 128).transpose(2, 1, 0).astype(bf)
        )
        in_maps.append(m)
    return in_maps


def kernel_run(inputs, trace=False):
    from concourse.bass_utils import run_bass_kernel_spmd

    nc = _get_nc()
    in_maps = _make_in_maps(inputs)
    res = run_bass_kernel_spmd(nc, in_maps, core_ids=list(range(N_CORES)), trace=trace)
    out = np.concatenate([r["score"] for r in res.results], axis=0)
    return out, res


def kernel(**inputs):
    out, _ = kernel_run(inputs)
    return out


# revision 7
# speedup vs baseline: 2.1048x; 1.0124x over previous
"""Trainium2 Bass kernel for nn_Attention_30760555774660 (stacked attention VQA net).

Sharding: data-parallel over batch, 256 -> 8 cores x 32. Weights replicated.

v3 design (per core: B=32, S=196, D=1024, A=512, O=3000):
  - All big matmuls in bf16 (tolerance gate is 2e-2; bf16 lands ~3e-3).
  - Host supplies img in BOTH layouts: native [b, s-chunks, d] for the
    attention-weighted sums, and pair-packed transposed [pair, d-part, c,
    392] for the projections (two batch elems side by side in the free dim
    so every projection matmul streams N=392).
  - Projections run transposed: projT[a, s2] = W_ia[:, a-chunk].T @ imgTP,
    W chunks stationary, out [128a, 392] PSUM (one bank per a-chunk).
  - q-projection broadcast + b_qa fold into tanh as the per-partition
    activation bias (QP1T/QP2T [a-part, b]).
  - logits = Wp.T @ haT on PE (M=1, N=392 per pair), softmax per b on one
    partition, E transposed back to [s, 1] via tiny PE transposes into a
    pre-masked [s, 8] group tile; vI for 8 b's accumulates into two
    [8, 512] PSUM banks.
  - u = vI*R + prev via one fused scalar_tensor_tensor per 512-chunk.
  - Final FC streams W_fc bf16 tiles (16 prefetched during the loop)
    against stationary u2T columns; b_fc folds in via a K=1 ones matmul.
"""

import sys

import numpy as np

if "/opt/trn_rl_repo" not in sys.path:
    sys.path.insert(0, "/opt/trn_rl_repo")

B_FULL = 256
N_CORES = 8
B = B_FULL // N_CORES  # 32
S = 196
S2 = 2 * S  # 392
D = 1024
A = 512
O = 3000
DC = 8  # d chunks of 128
AC = 4  # a chunks of 128
OB = 8  # batch group (oct)
NOCT = B // OB  # 4
NPAIR = B // 2  # 16
ON = 500
OC = 6
S_CHUNKS = ((0, 128), (1, 68))

_nc_cache = None


def _build_nc():
    import concourse.bacc as bacc
    import concourse.tile as tile
    from concourse import mybir

    f32 = mybir.dt.float32
    bf16 = mybir.dt.bfloat16
    Tanh = mybir.ActivationFunctionType.Tanh
    Exp = mybir.ActivationFunctionType.Exp
    mult = mybir.AluOpType.mult
    add = mybir.AluOpType.add

    nc = bacc.Bacc("TRN2", target_bir_lowering=False)

    imgN_h = nc.dram_tensor("imgN", [B, 128, 2, D], bf16, kind="ExternalInput")
    imgTP_h = nc.dram_tensor("imgTP", [NPAIR, 128, DC, S2], bf16, kind="ExternalInput")
    quesN_h = nc.dram_tensor("quesN", [B, D], f32, kind="ExternalInput")
    quesT_h = nc.dram_tensor("quesT", [128, DC, B], bf16, kind="ExternalInput")
    wia1_h = nc.dram_tensor("wia1", [128, DC, A], bf16, kind="ExternalInput")
    wia2_h = nc.dram_tensor("wia2", [128, DC, A], bf16, kind="ExternalInput")
    wqa1_h = nc.dram_tensor("wqa1", [128, DC, A], bf16, kind="ExternalInput")
    wqa2_h = nc.dram_tensor("wqa2", [128, DC, A], bf16, kind="ExternalInput")
    wfc_h = nc.dram_tensor("wfc", [128, DC, O], bf16, kind="ExternalInput")
    bfc_h = nc.dram_tensor("bfc", [1, O], bf16, kind="ExternalInput")
    wp1_h = nc.dram_tensor("wp1", [128, AC], bf16, kind="ExternalInput")
    wp2_h = nc.dram_tensor("wp2", [128, AC], bf16, kind="ExternalInput")
    bqa1T_h = nc.dram_tensor("bqa1T", [128, AC], f32, kind="ExternalInput")
    bqa2T_h = nc.dram_tensor("bqa2T", [128, AC], f32, kind="ExternalInput")
    onesb_h = nc.dram_tensor("onesb", [1, B], bf16, kind="ExternalInput")
    identf_h = nc.dram_tensor("identf", [128, 128], f32, kind="ExternalInput")
    identb_h = nc.dram_tensor("identb", [128, 128], bf16, kind="ExternalInput")
    score_h = nc.dram_tensor("score", [B, O], f32, kind="ExternalOutput")

    from contextlib import ExitStack

    with tile.TileContext(nc) as tc:
        with ExitStack() as stack:
            pool = lambda **kw: stack.enter_context(tc.tile_pool(**kw))
            const = pool(name="const", bufs=1)
            wqp = pool(name="wq", bufs=2)
            imgn_p = pool(name="imgn", bufs=11)
            imgt_p = pool(name="imgt", bufs=6)
            hap = pool(name="ha", bufs=4)
            parkp = pool(name="park", bufs=5)
            ep = pool(name="ep", bufs=4)
            etp = pool(name="etp", bufs=3)
            zp = pool(name="zp", bufs=4)
            rp = pool(name="rp", bufs=4)
            qpool = pool(name="qp", bufs=2)
            upool = pool(name="up", bufs=3)
            u1tp = pool(name="u1tp", bufs=2)
            qp2tp = pool(name="qp2tp", bufs=2)
            qpsp = pool(name="qps", bufs=2)
            wfp = pool(name="wf", bufs=20)
            scp = pool(name="sc", bufs=2)
            pps = pool(name="psproj", bufs=3, space="PSUM")
            pvi = pool(name="psvi", bufs=2, space="PSUM")
            psf = pool(name="pssmf", bufs=2, space="PSUM")
            psb = pool(name="pssmb", bufs=1, space="PSUM")
            wf_pre = []

            # ---- early constants (needed by the first projections) ----
            identf = const.tile([128, 128], f32, tag="identf")
            nc.sync.dma_start(out=identf, in_=identf_h[:, :])
            identb = const.tile([128, 128], bf16, tag="identb")
            nc.sync.dma_start(out=identb, in_=identb_h[:, :])
            wia1 = const.tile([128, DC, A], bf16, tag="wia1")
            nc.sync.dma_start(out=wia1, in_=wia1_h[:, :, :])
            wp1 = const.tile([128, AC], bf16, tag="wp1")
            nc.sync.dma_start(out=wp1, in_=wp1_h[:, :])
            bqa1T = const.tile([128, AC], f32, tag="bqa1T")
            nc.sync.dma_start(out=bqa1T, in_=bqa1T_h[:, :])
            quesT = const.tile([128, DC, B], bf16, tag="quesT")
            nc.sync.dma_start(out=quesT, in_=quesT_h[:, :, :])

            def load_oct_dmas(g):
                g8 = g * OB
                q8 = qpool.tile([OB, D], f32, tag="q8", name=f"q8_{g}")
                nc.sync.dma_start(out=q8, in_=quesN_h[g8 : g8 + OB, :])
                inbs = []
                itps = []
                for bb in range(OB):
                    b = g8 + bb
                    inb = imgn_p.tile([128, 2, D], bf16, tag="imgn", name=f"inb_{b}")
                    nc.sync.dma_start(
                        out=inb,
                        in_=imgN_h[b : b + 1, :, :, :].rearrange("o p k d -> (o p) k d"),
                    )
                    inbs.append(inb)
                for pp_ in range(OB // 2):
                    pr = g * (OB // 2) + pp_
                    itb = imgt_p.tile([128, DC, S2], bf16, tag="imgt", name=f"itp_{pr}")
                    nc.sync.dma_start(
                        out=itb,
                        in_=imgTP_h[pr : pr + 1, :, :, :].rearrange(
                            "o p c s -> (o p) c s"
                        ),
                    )
                    itps.append(itb)
                return q8, inbs, itps

            q8_g, inbs_g, itps_g = {}, {}, {}
            q8_g[0], inbs_g[0], itps_g[0] = load_oct_dmas(0)

            # ---- remaining constants ----
            onesb = const.tile([1, B], bf16, tag="onesb")
            nc.sync.dma_start(out=onesb, in_=onesb_h[:, :])
            wia2 = const.tile([128, DC, A], bf16, tag="wia2")
            nc.sync.dma_start(out=wia2, in_=wia2_h[:, :, :])
            wqa2 = const.tile([128, DC, A], bf16, tag="wqa2")
            nc.sync.dma_start(out=wqa2, in_=wqa2_h[:, :, :])
            wp2 = const.tile([128, AC], bf16, tag="wp2")
            nc.sync.dma_start(out=wp2, in_=wp2_h[:, :])
            bqa2T = const.tile([128, AC], f32, tag="bqa2T")
            nc.sync.dma_start(out=bqa2T, in_=bqa2T_h[:, :])
            bfc = const.tile([1, O], bf16, tag="bfc")
            nc.sync.dma_start(out=bfc, in_=bfc_h[:, :])
            QP1T = const.tile([128, AC, B], f32, tag="QP1T")
            u2T = const.tile([128, DC, B], bf16, tag="u2T")

            # ---------------- QP1T = (ques @ W_qa1 + b_qa1)^T ----------------
            qp1_ps = psf.tile([B, A], f32, tag="smf")
            for d in range(DC):
                wqs = wqp.tile([128, A], bf16, tag="wq")
                nc.sync.dma_start(out=wqs, in_=wqa1_h[:, d, :])
                nc.tensor.matmul(
                    qp1_ps, quesT[:, d, :], wqs, start=(d == 0), stop=(d == DC - 1)
                )
            qp1s = qpsp.tile([B, A], f32, tag="qps")
            nc.scalar.copy(qp1s, qp1_ps)
            for c in range(AC):
                pt = psf.tile([128, B], f32, tag="smf")
                nc.tensor.transpose(
                    pt, qp1s[:, c * 128 : (c + 1) * 128], identf[0:B, 0:B]
                )
                nc.vector.tensor_scalar_add(QP1T[:, c, :], pt, bqa1T[:, c : c + 1])

            def proj_tanh(itb, wia, QPT, bias_cols, out_ha, parked):
                """Pair projection + tanh (or park copy) per a-chunk.

                out_ha: [128, AC, S2] bf16 target; bias_cols: (col0, col1) into
                QPT for the two batch elems, or None to park (plain copy)."""
                for a in range(AC):
                    ppt = pps.tile([128, S2], f32, tag="proj", name=f"pj_{id(out_ha)}_{a}")
                    for d in range(DC):
                        nc.tensor.matmul(
                            ppt,
                            wia[:, d, a * 128 : (a + 1) * 128],
                            itb[:, d, :],
                            start=(d == 0),
                            stop=(d == DC - 1),
                        )
                    if parked:
                        nc.scalar.copy(out_ha[:, a, :], ppt)
                    else:
                        for h in range(2):
                            nc.scalar.activation(
                                out_ha[:, a, h * S : (h + 1) * S],
                                ppt[:, h * S : (h + 1) * S],
                                Tanh,
                                bias=QPT[:, a, bias_cols[h] : bias_cols[h] + 1],
                            )

            def tanh_parked(pk, QPT, bias_cols, out_ha):
                for a in range(AC):
                    for h in range(2):
                        nc.scalar.activation(
                            out_ha[:, a, h * S : (h + 1) * S],
                            pk[:, a, h * S : (h + 1) * S],
                            Tanh,
                            bias=QPT[:, a, bias_cols[h] : bias_cols[h] + 1],
                        )

            def logits_softmax(ha, wp, et, z, bb0):
                """Pair logits -> per-b exp -> E^T columns into group tile."""
                lg = psf.tile([1, S2], f32, tag="smf", name=f"lg_{id(ha)}")
                for c in range(AC):
                    nc.tensor.matmul(
                        lg, wp[:, c : c + 1], ha[:, c, :], start=(c == 0), stop=(c == AC - 1)
                    )
                for h in range(2):
                    bb = bb0 + h
                    E = ep.tile([1, S], bf16, tag="E", name=f"E_{id(ha)}_{h}")
                    nc.scalar.activation(
                        E, lg[0:1, h * S : (h + 1) * S], Exp,
                        accum_out=z[0:1, bb : bb + 1],
                    )
                    for si, sl in S_CHUNKS:
                        pt = psb.tile([128, 1], bf16, tag="smb", name=f"pt_{id(ha)}_{h}_{si}")
                        nc.tensor.transpose(
                            pt[0:sl, :], E[0:1, si * 128 : si * 128 + sl],
                            identb[0:1, 0:1],
                        )
                        nc.vector.tensor_copy(et[0:sl, si, bb, bb : bb + 1], pt[0:sl, :])

            def group_vI_u(et, z, inbs, other, nm):
                """vI for 8 b's + fused u = vI*R + other. Returns u [8,1024] f32."""
                ztp = psf.tile([OB, 1], f32, tag="smf", name=f"ztp_{nm}")
                nc.tensor.transpose(ztp[0:OB, :], z[0:1, 0:OB], identf[0:1, 0:1])
                R = rp.tile([OB, 1], f32, tag="R", name=f"R_{nm}")
                nc.vector.reciprocal(R, ztp[0:OB, :])
                vis = [
                    pvi.tile([OB, A], f32, tag="vi", name=f"vi_{nm}_0"),
                    pvi.tile([OB, A], f32, tag="vi", name=f"vi_{nm}_1"),
                ]
                for n in range(2):
                    for bb in range(OB):
                        for si, sl in S_CHUNKS:
                            nc.tensor.matmul(
                                vis[n],
                                et[0:sl, si, bb, :],
                                inbs[bb][0:sl, si, n * A : (n + 1) * A],
                                start=(bb == 0 and si == 0),
                                stop=(bb == OB - 1 and si == 1),
                            )
                u = upool.tile([OB, D], f32, tag="u", name=f"u_{nm}")
                for n in range(2):
                    nc.vector.scalar_tensor_tensor(
                        u[0:OB, n * A : (n + 1) * A],
                        vis[n],
                        R[0:OB, 0:1],
                        other[0:OB, n * A : (n + 1) * A],
                        op0=mult,
                        op1=add,
                    )
                return u

            def transpose_cols(u, dst_ap_fn, nm):
                """u [8, 1024] f32 -> 8 chunk transposes into dst [128, c, 8]."""
                for c in range(DC):
                    pt = psf.tile([128, OB], f32, tag="smf", name=f"ut_{nm}_{c}")
                    nc.tensor.transpose(
                        pt[:, 0:OB], u[0:OB, c * 128 : (c + 1) * 128], identf[0:OB, 0:OB]
                    )
                    nc.vector.tensor_copy(dst_ap_fn(c), pt[:, 0:OB])

            # ---------------- main loop over octs ----------------
            for g in range(NOCT):
                g8 = g * OB
                if g not in q8_g:
                    q8_g[g], inbs_g[g], itps_g[g] = load_oct_dmas(g)
                q8, inbs, itps = q8_g[g], inbs_g[g], itps_g[g]
                # prefetch next oct's DMAs early
                if g + 1 < NOCT:
                    q8_g[g + 1], inbs_g[g + 1], itps_g[g + 1] = load_oct_dmas(g + 1)
                # prefetch W_fc tiles during the last two octs
                if g >= 2:
                    for j in range(8):
                        wf = wfp.tile([128, ON], bf16, tag="wf", name=f"wf_{g}_{j}")
                        k = (g - 2) * 8 + j
                        n, c = divmod(k, DC)
                        nc.sync.dma_start(out=wf, in_=wfc_h[:, c, n * ON : (n + 1) * ON])
                        wf_pre.append(wf)

                # block 1 per-pair: proj -> tanh -> logits -> exp -> E^T
                et1 = etp.tile([128, 2, OB, OB], bf16, tag="et", name=f"et1_{g}")
                nc.vector.memset(et1, 0.0)
                z1 = zp.tile([1, OB], f32, tag="z", name=f"z1_{g}")
                for pp_ in range(OB // 2):
                    ha = hap.tile([128, AC, S2], bf16, tag="ha", name=f"ha1_{g}_{pp_}")
                    proj_tanh(
                        itps[pp_], wia1, QP1T,
                        (g8 + 2 * pp_, g8 + 2 * pp_ + 1), ha, parked=False,
                    )
                    logits_softmax(ha, wp1, et1, z1, 2 * pp_)

                # block 2 projections parked to SBUF (independent of u1)
                parks = []
                for pp_ in range(OB // 2):
                    pk = parkp.tile([128, AC, S2], bf16, tag="park", name=f"pk_{g}_{pp_}")
                    proj_tanh(itps[pp_], wia2, None, None, pk, parked=True)
                    parks.append(pk)

                # u1 = vI1 + ques
                u1 = group_vI_u(et1, z1, inbs, q8, f"u1_{g}")

                # u1T (bf16) for the QP2 matvec
                u1T = u1tp.tile([128, DC, OB], bf16, tag="u1T", name=f"u1T_{g}")
                transpose_cols(u1, lambda c: u1T[:, c, :], f"u1_{g}")

                # QP2T = (u1 @ W_qa2 + b_qa2)^T
                qp2_ps = psf.tile([OB, A], f32, tag="smf", name=f"qp2ps_{g}")
                for d in range(DC):
                    nc.tensor.matmul(
                        qp2_ps, u1T[:, d, :], wqa2[:, d, :],
                        start=(d == 0), stop=(d == DC - 1),
                    )
                qp2s = qpsp.tile([OB, A], f32, tag="qps", name=f"qp2s_{g}")
                nc.scalar.copy(qp2s, qp2_ps)
                QP2T = qp2tp.tile([128, AC, OB], f32, tag="qp2T", name=f"QP2T_{g}")
                for c in range(AC):
                    pt = psf.tile([128, OB], f32, tag="smf", name=f"qt_{g}_{c}")
                    nc.tensor.transpose(
                        pt[:, 0:OB], qp2s[0:OB, c * 128 : (c + 1) * 128],
                        identf[0:OB, 0:OB],
                    )
                    nc.vector.tensor_scalar_add(
                        QP2T[:, c, :], pt[:, 0:OB], bqa2T[:, c : c + 1]
                    )

                # block 2 per-pair tail
                et2 = etp.tile([128, 2, OB, OB], bf16, tag="et", name=f"et2_{g}")
                nc.vector.memset(et2, 0.0)
                z2 = zp.tile([1, OB], f32, tag="z", name=f"z2_{g}")
                for pp_ in range(OB // 2):
                    ha2 = hap.tile([128, AC, S2], bf16, tag="ha", name=f"ha2_{g}_{pp_}")
                    tanh_parked(parks[pp_], QP2T, (2 * pp_, 2 * pp_ + 1), ha2)
                    logits_softmax(ha2, wp2, et2, z2, 2 * pp_)

                # u2 = vI2 + u1
                u2 = group_vI_u(et2, z2, inbs, u1, f"u2_{g}")
                transpose_cols(u2, lambda c: u2T[:, c, g8 : g8 + OB], f"u2_{g}")

            # ---------------- final FC ----------------
            for n in range(OC):
                wfn = []
                for c in range(DC):
                    k = n * DC + c
                    if k < len(wf_pre):
                        wfn.append(wf_pre[k])
                    else:
                        wf = wfp.tile([128, ON], bf16, tag="wf", name=f"wfl_{n}_{c}")
                        nc.sync.dma_start(out=wf, in_=wfc_h[:, c, n * ON : (n + 1) * ON])
                        wfn.append(wf)
                sp = pps.tile([B, ON], f32, tag="proj", name=f"sp_{n}")
                for c in range(DC):
                    nc.tensor.matmul(sp, u2T[:, c, :], wfn[c], start=(c == 0), stop=False)
                nc.tensor.matmul(
                    sp, onesb[0:1, :], bfc[0:1, n * ON : (n + 1) * ON],
                    start=False, stop=True,
                )
                sc = scp.tile([B, ON], f32, tag="sc", name=f"sc_{n}")
                nc.scalar.copy(sc, sp)
                nc.sync.dma_start(out=score_h[:, n * ON : (n + 1) * ON], in_=sc)

    nc.compile()
    return nc


def _get_nc():
    global _nc_cache
    if _nc_cache is None:
        _nc_cache = _build_nc()
    return _nc_cache


def _make_in_maps(inputs):
    import ml_dtypes

    bf = ml_dtypes.bfloat16

    def f32a(x):
        return np.ascontiguousarray(np.asarray(x), np.float32)

    def wchunk(w):  # [D, N] -> [128, DC, N]
        w = f32a(w)
        return np.ascontiguousarray(
            w.reshape(DC, 128, w.shape[1]).transpose(1, 0, 2).astype(bf)
        )

    def acolT(v, dt):  # [A] -> [128, AC]
        return np.ascontiguousarray(f32a(v).reshape(AC, 128).T.astype(dt))

    img = f32a(inputs["img_feat"])  # [256, 196, 1024]
    ques = f32a(inputs["ques_feat"])  # [256, 1024]

    imgN = np.zeros((B_FULL, 128, 2, D), np.float32)
    imgN[:, :, 0, :] = img[:, 0:128, :]
    imgN[:, 0:68, 1, :] = img[:, 128:196, :]
    imgN = np.ascontiguousarray(imgN.astype(bf))
    # [B, 128, DC, S] then pack pairs along the last axis -> [B//2, 128, DC, 392]
    imgT = img.reshape(B_FULL, S, DC, 128).transpose(0, 3, 2, 1)
    imgTP = np.concatenate(
        [imgT[0::2], imgT[1::2]], axis=3
    )  # [128 pairs, 128, DC, 392]
    imgTP = np.ascontiguousarray(imgTP.astype(bf))

    shared = {
        "wia1": wchunk(inputs["W_ia1"]),
        "wia2": wchunk(inputs["W_ia2"]),
        "wqa1": wchunk(inputs["W_qa1"]),
        "wqa2": wchunk(inputs["W_qa2"]),
        "wfc": wchunk(inputs["W_fc"]),
        "bfc": np.ascontiguousarray(f32a(inputs["b_fc"]).reshape(1, O).astype(bf)),
        "wp1": acolT(inputs["Wp1"], bf),
        "wp2": acolT(inputs["Wp2"], bf),
        "bqa1T": acolT(inputs["b_qa1"], np.float32),
        "bqa2T": acolT(inputs["b_qa2"], np.float32),
        "onesb": np.ones((1, B), bf),
        "identf": np.eye(128, dtype=np.float32),
        "identb": np.eye(128, dtype=np.float32).astype(bf),
    }
    in_maps = []
    for core in range(N_CORES):
        sl = slice(core * B, (core + 1) * B)
        slp = slice(core * NPAIR, (core + 1) * NPAIR)
        m = dict(shared)
        m["imgN"] = imgN[sl]
        m["imgTP"] = imgTP[slp]
        m["quesN"] = np.ascontiguousarray(ques[sl])
        m["quesT"] = np.ascontiguousarray(
            ques[sl].reshape(B, DC, 128).transpose(2, 1, 0).astype(bf)
        )
        in_maps.append(m)
    return in_maps


def kernel_run(inputs, trace=False):
    from concourse.bass_utils import run_bass_kernel_spmd

    nc = _get_nc()
    in_maps = _make_in_maps(inputs)
    res = run_bass_kernel_spmd(nc, in_maps, core_ids=list(range(N_CORES)), trace=trace)
    out = np.concatenate([r["score"] for r in res.results], axis=0)
    return out, res


def kernel(**inputs):
    out, _ = kernel_run(inputs)
    return out
